# revision 36
# baseline (speedup 1.0000x reference)
"""Trainium2 Bass kernel for nn_CFHoTWrapper (sparse attention with adapter gate).

Sharding: tensor-parallel over attention heads across 8 NeuronCores.
Each core computes 4 query heads + its 1 KV head end-to-end (QKV proj,
RoPE, scores, softmax, AV, partial O-projection); the tiny adapter gate
is replicated on every core. Per-core partial outputs (bf16) are summed
on the host.

Softmax is computed without max-subtraction (scores are O(5) for these
shapes so exp() is safe in fp32), and the per-key gate bias is folded in
multiplicatively: exp(s + m + g[k]) = exp(s) * exp(m) * w[k] with
w = exp(gate_scale * gate).  w scales the V rows, and an extra all-w
column appended to V yields the softmax denominator from the same
matmul that computes the numerator.
"""

import math
import os
from contextlib import ExitStack

import numpy as np
import ml_dtypes

import concourse.bass as bass
import concourse.tile as tile
from concourse import mybir
from concourse.masks import make_identity
from concourse.bass_utils import run_bass_kernel_spmd

BF16 = ml_dtypes.bfloat16
F32 = np.float32

S = 2048
D = 2048
HD = 64
NH = 32
NKV = 8
NCORES = 8
HLOC = NH // NCORES          # 4 query heads per core
P = 128
NT = S // P                  # 16 sequence tiles of 128
NCH = 4                      # 4 sequence chunks of 512
CH = 512
ALPHA = 0.995
MASK_NEG_THRESH = -80.0      # exp() underflows to 0 below this

LAST_RESULT = None           # BassKernelResults of the last run (for test.py)


def _analyze_mask(maskT):
    """Classify [keys=128 x q=128] blocks of maskT and dedup non-trivial
    multiplicative (exp) mask patterns. maskT is [S, S] (keys, q).

    Returns:
      mb:       [NT][NT] block class: 'skip' | 'plain' | int pattern id
      patterns: list of [128, 128] bf16 multiplicative masks
      av_incl:  per q-tile i, the key-tiles j contributing to softmax/AV
    """
    mb = [[None] * NT for _ in range(NT)]
    patterns = []
    pat_index = {}
    for j in range(NT):
        for i in range(NT):
            blk = maskT[j * P:(j + 1) * P, i * P:(i + 1) * P]
            if (blk < MASK_NEG_THRESH).all():
                mb[j][i] = 'skip'
            elif (blk == 0.0).all():
                mb[j][i] = 'plain'
            else:
                pat = np.exp(np.minimum(blk, 80.0)).astype(BF16)
                key = pat.tobytes()
                if key not in pat_index:
                    pat_index[key] = len(patterns)
                    patterns.append(pat)
                mb[j][i] = pat_index[key]
    av_incl = [[j for j in range(NT) if mb[j][i] != 'skip'] for i in range(NT)]
    return mb, patterns, av_incl


def _split_sync_waits(nc):
    """This walrus build supports only ONE embedded sync wait per
    instruction; hoist extra waits onto preceding sequencer NoOps."""
    for f in nc.m.functions:
        for bb in f.blocks:
            insts = bb.instructions
            idx = 0
            while idx < len(insts):
                inst = insts[idx]
                si = inst.sync_info
                if si is not None and si.on_wait and len(si.on_wait) > 1:
                    waits = list(si.on_wait)
                    for w in waits[:-1]:
                        nop = mybir.InstNoOp(
                            name=nc.get_next_instruction_name(),
                            engine=inst.engine,
                            sync_info=mybir.SyncInfo(on_wait=[w], on_update=[]),
                            bass_nofuse=True,
                        )
                        nc.register_instruction(nop)
                        insts.insert(idx, nop)
                        idx += 1
                    inst.sync_info = mybir.SyncInfo(
                        on_wait=[waits[-1]], on_update=list(si.on_update))
                idx += 1


def _build_program(mb, n_pat, av_incl, field_scale, b2_scaled, gate_scale,
                   erf_fn=None, debug=False):
    nc = bass.Bass()
    dt = mybir.dt
    if erf_fn is None:
        erf_fn = mybir.ActivationFunctionType.Erf
    dbg = {}
    if debug:
        dbg["qt0"] = nc.declare_dram_parameter("d_qt0", [HD, S], dt.bfloat16, isOutput=True)
        dbg["kt"] = nc.declare_dram_parameter("d_kt", [HD, S], dt.bfloat16, isOutput=True)
        dbg["vaug"] = nc.declare_dram_parameter("d_vaug", [P, NT * (HD + 1)], dt.bfloat16, isOutput=True)
        dbg["wcol"] = nc.declare_dram_parameter("d_wcol", [P, NT], dt.float32, isOutput=True)
        dbg["gate"] = nc.declare_dram_parameter("d_gate", [1, S], dt.float32, isOutput=True)
        dbg["field"] = nc.declare_dram_parameter("d_field", [1, S], dt.float32, isOutput=True)
        dbg["hmT"] = nc.declare_dram_parameter("d_hmT", [64, S], dt.bfloat16, isOutput=True)
        dbg["fibT"] = nc.declare_dram_parameter("d_fibT", [16, S], dt.bfloat16, isOutput=True)
        dbg["attn"] = nc.declare_dram_parameter("d_attn", [P, NT * HLOC * HD], dt.bfloat16, isOutput=True)
        dbg["pt00"] = nc.declare_dram_parameter("d_pt00", [P, CH], dt.bfloat16, isOutput=True)
        dbg["pair0"] = nc.declare_dram_parameter("d_pair0", [P, S], dt.bfloat16, isOutput=True)

    hT_d = nc.declare_dram_parameter("hT", [D, S], dt.bfloat16, isOutput=False)
    wq_d = nc.declare_dram_parameter("wq", [D, HLOC * HD], dt.bfloat16, isOutput=False)
    wk_d = nc.declare_dram_parameter("wk", [D, HD], dt.bfloat16, isOutput=False)
    wv_d = nc.declare_dram_parameter("wv", [D, HD], dt.bfloat16, isOutput=False)
    wo_d = nc.declare_dram_parameter("wo", [HLOC * HD, D], dt.bfloat16, isOutput=False)
    w1a_d = nc.declare_dram_parameter("w1a", [D, 64], dt.bfloat16, isOutput=False)
    w1b_d = nc.declare_dram_parameter("w1b", [16, 64], dt.bfloat16, isOutput=False)
    wf_d = nc.declare_dram_parameter("wf", [D, 16], dt.bfloat16, isOutput=False)
    w2_d = nc.declare_dram_parameter("w2", [64, 1], dt.bfloat16, isOutput=False)
    b1_d = nc.declare_dram_parameter("b1", [64, 1], dt.float32, isOutput=False)
    cosq_d = nc.declare_dram_parameter("cosq", [HD, S], dt.bfloat16, isOutput=False)
    sinq_d = nc.declare_dram_parameter("sinq", [HD, S], dt.bfloat16, isOutput=False)
    cosk_d = nc.declare_dram_parameter("cosk", [HD, S], dt.bfloat16, isOutput=False)
    sink_d = nc.declare_dram_parameter("sink", [HD, S], dt.bfloat16, isOutput=False)
    if n_pat:
        pm_d = nc.declare_dram_parameter("pmask", [n_pat, P, P], dt.bfloat16, isOutput=False)
    out_d = nc.declare_dram_parameter("out", [S, D], dt.bfloat16, isOutput=True)

    with tile.TileContext(nc) as tc, ExitStack() as ctx:
        pers = ctx.enter_context(tc.tile_pool(name="pers", bufs=1))
        psp = ctx.enter_context(tc.tile_pool(name="psum", bufs=4, space="PSUM"))

        # ---------------- persistent loads ----------------
        # small adapter weights first so the PE can start within ~4us;
        # hT streams per-chunk (consumers unblock chunk by chunk); wo last
        # (only needed by the output projection).
        wf = pers.tile([P, NT, 16], dt.bfloat16)
        nc.sync.dma_start(out=wf, in_=wf_d.rearrange("(k p) f -> p k f", p=P))
        w1a = pers.tile([P, NT, 64], dt.bfloat16)
        nc.sync.dma_start(out=w1a, in_=w1a_d.rearrange("(k p) f -> p k f", p=P))
        w1b = pers.tile([16, 64], dt.bfloat16)
        nc.sync.dma_start(out=w1b, in_=w1b_d[:, :])
        w2 = pers.tile([64, 1], dt.bfloat16)
        nc.sync.dma_start(out=w2, in_=w2_d[:, :])
        b1 = pers.tile([64, 1], dt.float32)
        nc.sync.dma_start(out=b1, in_=b1_d[:, :])
        hT = pers.tile([P, NT, S], dt.bfloat16)
        for k in range(NT):
            nc.sync.dma_start(out=hT[:, k, :], in_=hT_d[k * P:(k + 1) * P, :])
        wq = pers.tile([P, NT, HLOC * HD], dt.bfloat16)
        nc.sync.dma_start(out=wq, in_=wq_d.rearrange("(k p) f -> p k f", p=P))
        wk = pers.tile([P, NT, HD], dt.bfloat16)
        nc.sync.dma_start(out=wk, in_=wk_d.rearrange("(k p) f -> p k f", p=P))
        wv = pers.tile([P, NT, HD], dt.bfloat16)
        nc.sync.dma_start(out=wv, in_=wv_d.rearrange("(k p) f -> p k f", p=P))
        if n_pat:
            pmask = pers.tile([P, n_pat, P], dt.bfloat16)
            for m in range(n_pat):
                nc.sync.dma_start(out=pmask[:, m, :], in_=pm_d[m, :, :])
        wo = pers.tile([P, 2, D], dt.bfloat16)
        nc.sync.dma_start(out=wo, in_=wo_d.rearrange("(g p) d -> p g d", p=P))
        ident = pers.tile([P, P], dt.bfloat16)
        make_identity(nc, ident)
        ones64 = pers.tile([1, HD], dt.float32)
        nc.vector.memset(ones64, 1.0)

        qt = [pers.tile([HD, S], dt.bfloat16, tag=f"qt{h}", name=f"qt{h}")
              for h in range(HLOC)]
        kt = pers.tile([HD, S], dt.bfloat16)
        vaug = pers.tile([P, NT, HD + 1], dt.bfloat16)
        wcol = pers.tile([P, NT], dt.float32)

        # ---------------- phase A+B: adapter gate & QKV projections ----------
        with tc.tile_pool(name="phb", bufs=1) as phb, \
             tc.tile_pool(name="phbw", bufs=2) as phbw:
            cosq = phb.tile([HD, S], dt.bfloat16)
            nc.sync.dma_start(out=cosq, in_=cosq_d[:, :])
            sinq = phb.tile([HD, S], dt.bfloat16)
            nc.sync.dma_start(out=sinq, in_=sinq_d[:, :])
            cosk = phb.tile([HD, S], dt.bfloat16)
            nc.sync.dma_start(out=cosk, in_=cosk_d[:, :])
            sink = phb.tile([HD, S], dt.bfloat16)
            nc.sync.dma_start(out=sink, in_=sink_d[:, :])

            # --- adapter: fiberT = Wf^T @ hT ---
            fibT = phb.tile([16, S], dt.bfloat16)
            for c in range(NCH):
                ps = psp.tile([16, CH], dt.float32, tag="sc", bufs=4)
                for k in range(NT):
                    nc.tensor.matmul(ps, wf[:, k, :], hT[:, k, c * CH:(c + 1) * CH],
                                     start=(k == 0), stop=(k == NT - 1))
                nc.vector.tensor_copy(fibT[:, c * CH:(c + 1) * CH], ps)
            # --- hmidT = gelu(W1^T @ [hT; fibT] + b1) ---
            hmT = phb.tile([64, S], dt.bfloat16)
            for c in range(NCH):
                ps = psp.tile([64, CH], dt.float32, tag="sc", bufs=4)
                for k in range(NT):
                    nc.tensor.matmul(ps, w1a[:, k, :], hT[:, k, c * CH:(c + 1) * CH],
                                     start=(k == 0), stop=False)
                nc.tensor.matmul(ps, w1b, fibT[:, c * CH:(c + 1) * CH],
                                 start=False, stop=True)
                # exact gelu(x) = 0.5 * x * (1 + erf(x / sqrt(2))), x = ps + b1
                pre = phbw.tile([64, CH], dt.float32, tag="pre")
                nc.vector.tensor_scalar(pre, ps, b1, None, mybir.AluOpType.add)
                er = phbw.tile([64, CH], dt.float32, tag="er")
                nc.scalar.activation(er, pre, erf_fn,
                                     bias=0.0, scale=1.0 / math.sqrt(2.0))
                nc.vector.tensor_scalar(er, er, 0.5, 0.5,
                                        mybir.AluOpType.mult, mybir.AluOpType.add)
                nc.vector.tensor_mul(hmT[:, c * CH:(c + 1) * CH], pre, er)
            # --- field row = field_scale * (hmidT^T @ W2 + b2) ---
            field = phb.tile([1, S], dt.float32)
            scratch = phb.tile([1, S], dt.float32)
            for c in range(NCH):
                ps = psp.tile([1, CH], dt.float32, tag="sc", bufs=4)
                nc.tensor.matmul(ps, w2, hmT[:, c * CH:(c + 1) * CH],
                                 start=True, stop=True)
                nc.vector.tensor_scalar(field[:, c * CH:(c + 1) * CH], ps,
                                        field_scale, b2_scaled,
                                        mybir.AluOpType.mult, mybir.AluOpType.add)
            # --- standardize: gate = (field - mean) / (std_ddof1 + 1e-6) ---
            ssum = phb.tile([1, 1], dt.float32)
            nc.vector.reduce_sum(ssum, field, axis=mybir.AxisListType.X)
            mean = phb.tile([1, 1], dt.float32)
            nc.vector.tensor_scalar_mul(mean, ssum, 1.0 / S)
            nc.vector.tensor_scalar(field, field, mean, None, mybir.AluOpType.subtract)
            nc.scalar.square(scratch, field)
            ss2 = phb.tile([1, 1], dt.float32)
            nc.vector.reduce_sum(ss2, scratch, axis=mybir.AxisListType.X)
            std = phb.tile([1, 1], dt.float32)
            nc.scalar.activation(std, ss2, mybir.ActivationFunctionType.Sqrt,
                                 bias=0.0, scale=1.0 / (S - 1))
            nc.vector.tensor_scalar_add(std, std, 1e-6)
            rstd = phb.tile([1, 1], dt.float32)
            nc.vector.reciprocal(rstd, std)
            gsr = phb.tile([1, 1], dt.float32)
            nc.vector.tensor_scalar_mul(gsr, rstd, gate_scale)
            # w row = exp(gate_scale * gate), into scratch
            nc.scalar.activation(scratch, field, mybir.ActivationFunctionType.Exp,
                                 bias=0.0, scale=gsr)
            # transpose the w row into per-partition columns [128, 16] via a
            # DRAM bounce (SBUF partitions are not element-addressable across
            # the partition stride, so an in-SBUF gather is illegal on HW)
            wrow_dram = nc.dram_tensor("wrow_dram", [1, S], dt.float32)
            nc.sync.dma_start(out=wrow_dram[:, :], in_=scratch)
            nc.sync.dma_start(out=wcol,
                              in_=wrow_dram[0, :].rearrange("(j p) -> p j", p=P))
            if debug:
                nc.sync.dma_start(out=dbg["gate"][:, :], in_=scratch)
                nc.sync.dma_start(out=dbg["field"][:, :], in_=field)
                nc.sync.dma_start(out=dbg["hmT"][:, :], in_=hmT)
                nc.sync.dma_start(out=dbg["fibT"][:, :], in_=fibT)
                nc.sync.dma_start(out=dbg["wcol"][:, :], in_=wcol)

            # --- Q projection (head pairs) + RoPE ---
            # Compute engines require matching base partitions on SBUF
            # operands, so all partition moves (head extraction, the
            # rotate-half swap) go through SBUF->SBUF DMA.
            def rope(raw, swp, cos_t, sin_t, out_ap):
                t1 = phbw.tile([HD, S], dt.bfloat16, tag="t1")
                nc.vector.tensor_mul(t1, raw, cos_t)
                t2 = phbw.tile([HD, S], dt.bfloat16, tag="t2")
                nc.vector.tensor_mul(t2, swp, sin_t)
                nc.vector.tensor_add(out_ap, t1, t2)

            for t in range(HLOC // 2):
                pair = phbw.tile([P, S], dt.bfloat16, tag="qpair")
                for c in range(NCH):
                    ps = psp.tile([P, CH], dt.float32, tag="sc", bufs=4)
                    for k in range(NT):
                        nc.tensor.matmul(ps, wq[:, k, t * P:(t + 1) * P],
                                         hT[:, k, c * CH:(c + 1) * CH],
                                         start=(k == 0), stop=(k == NT - 1))
                    nc.vector.tensor_copy(pair[:, c * CH:(c + 1) * CH], ps)
                for hh in range(2):
                    h = 2 * t + hh
                    raw = phbw.tile([HD, S], dt.bfloat16, tag="qraw")
                    nc.sync.dma_start(out=raw, in_=pair[hh * HD:(hh + 1) * HD, :])
                    swp = phbw.tile([HD, S], dt.bfloat16, tag="qswp")
                    nc.sync.dma_start(out=swp[0:32, :],
                                      in_=pair[hh * HD + 32:hh * HD + 64, :])
                    nc.sync.dma_start(out=swp[32:64, :],
                                      in_=pair[hh * HD:hh * HD + 32, :])
                    rope(raw, swp, cosq, sinq, qt[h][:, :])
                    if debug and h == 0:
                        nc.sync.dma_start(out=dbg["pair0"][:, :], in_=pair)
                        nc.sync.dma_start(out=dbg["qt0"][:, :], in_=qt[0])
            # --- K projection + RoPE ---
            kraw = phbw.tile([HD, S], dt.bfloat16, tag="qraw")
            for c in range(NCH):
                ps = psp.tile([HD, CH], dt.float32, tag="sc", bufs=4)
                for k in range(NT):
                    nc.tensor.matmul(ps, wk[:, k, :], hT[:, k, c * CH:(c + 1) * CH],
                                     start=(k == 0), stop=(k == NT - 1))
                nc.vector.tensor_copy(kraw[:, c * CH:(c + 1) * CH], ps)
            kswp = phbw.tile([HD, S], dt.bfloat16, tag="qswp")
            nc.sync.dma_start(out=kswp[0:32, :], in_=kraw[32:64, :])
            nc.sync.dma_start(out=kswp[32:64, :], in_=kraw[0:32, :])
            rope(kraw, kswp, cosk, sink, kt[:, :])
            if debug:
                nc.sync.dma_start(out=dbg["kt"][:, :], in_=kt)
            # --- V projection (transposed form) + gate scaling + denom col ---
            vt = phb.tile([HD, S], dt.bfloat16)
            for c in range(NCH):
                ps = psp.tile([HD, CH], dt.float32, tag="sc", bufs=4)
                for k in range(NT):
                    nc.tensor.matmul(ps, wv[:, k, :], hT[:, k, c * CH:(c + 1) * CH],
                                     start=(k == 0), stop=(k == NT - 1))
                nc.vector.tensor_copy(vt[:, c * CH:(c + 1) * CH], ps)
            for st in range(NT):
                pv = psp.tile([P, HD], dt.bfloat16, tag="od", bufs=2)
                nc.tensor.transpose(pv, vt[:, st * P:(st + 1) * P],
                                    ident[0:HD, 0:HD])
                nc.vector.tensor_scalar(vaug[:, st, 0:HD], pv, wcol[:, st:st + 1],
                                        None, mybir.AluOpType.mult)
                nc.vector.tensor_copy(vaug[:, st, HD:HD + 1], wcol[:, st:st + 1])
            if debug:
                nc.sync.dma_start(
                    out=dbg["vaug"][:, :],
                    in_=vaug.rearrange("p a b -> p (a b)"))

        # ------------- phase C: attention + output projection -------------
        with tc.tile_pool(name="phc", bufs=2) as phc, \
             tc.tile_pool(name="phcs", bufs=4) as phcs, \
             tc.tile_pool(name="phd", bufs=3) as phd:

            # per-(j, chunk): first q-column (within the chunk) that isn't
            # fully masked -> scores MM / exp start column; None = skip tile
            def chunk_start(j, c):
                for ii in range(4 * c, 4 * c + 4):
                    if mb[j][ii] != 'skip':
                        return (ii % 4) * P
                return None

            pts_store = {}
            aT_tiles = {}

            def emit_scores(c, h):
                pts = {}
                for j in range(NT):
                    s0 = chunk_start(j, c)
                    if s0 is None:
                        continue
                    ps = psp.tile([P, CH], dt.float32, tag="sc", bufs=4,
                                  name="ps_sc")
                    nc.tensor.matmul(ps[:, s0:CH], kt[:, j * P:(j + 1) * P],
                                     qt[h][:, c * CH + s0:(c + 1) * CH],
                                     start=True, stop=True)
                    pt = phc.tile([P, CH], dt.bfloat16, tag=f"pt{j}",
                                  name=f"pt{j}")
                    nc.scalar.activation(pt[:, s0:CH], ps[:, s0:CH],
                                         mybir.ActivationFunctionType.Exp)
                    if s0 > 0:
                        nc.vector.memset(pt[:, 0:s0], 0.0)
                    for ii in range(4 * c, 4 * c + 4):
                        kind = mb[j][ii]
                        qq = slice((ii % 4) * P, (ii % 4 + 1) * P)
                        if kind == 'plain':
                            continue
                        if kind == 'skip':
                            # non-leading masked block (generic masks only):
                            # zero it so the accumulated AV stays exact
                            if (ii % 4) * P > s0:
                                nc.vector.memset(pt[:, qq], 0.0)
                            continue
                        nc.vector.tensor_mul(pt[:, qq], pt[:, qq],
                                             pmask[:, kind, :])
                    pts[j] = pt
                    if debug and h == 0 and c == 0 and j == 0:
                        nc.sync.dma_start(out=dbg["pt00"][:, :], in_=pt)
                pts_store[(c, h)] = pts

            def emit_av(c, h):
                """Transposed AV: avT[hd+denom, q] = vaug^T @ p^T, then divide
                by the broadcast denominator row; lands pre-transposed for the
                output projection in a head-pair tile."""
                pts = pts_store.pop((c, h))
                t, hh = h // 2, h % 2
                if (t, c) not in aT_tiles:
                    aT_tiles[(t, c)] = phc.tile(
                        [P, CH], dt.bfloat16, tag=f"aT{t}", name=f"aT{t}")
                aT = aT_tiles[(t, c)]
                rows = slice(hh * HD, (hh + 1) * HD)
                js = sorted(pts.keys())
                if not js:
                    nc.vector.memset(aT[rows, :], 0.0)
                    return
                pa = psp.tile([HD + 1, CH], dt.float32, tag="av", bufs=2,
                              name="ps_avT")
                for idx, j in enumerate(js):
                    nc.tensor.matmul(pa, vaug[:, j, :], pts[j],
                                     start=(idx == 0), stop=(idx == len(js) - 1))
                den = phcs.tile([1, CH], dt.float32, tag="den", name="den")
                nc.vector.tensor_copy(den, pa[HD:HD + 1, :])
                rb = psp.tile([HD, CH], dt.float32, tag="od", bufs=2,
                              name="ps_rb")
                nc.tensor.matmul(rb, ones64, den, start=True, stop=True)
                rcb = phcs.tile([HD, CH], dt.float32, tag="rcb", name="rcb")
                nc.vector.reciprocal(rcb, rb)
                numT = phcs.tile([HD, CH], dt.bfloat16, tag="numT", name="numT")
                nc.vector.tensor_copy(numT, pa[0:HD, :])
                nc.vector.tensor_mul(aT[rows, :], numT, rcb)

            def emit_oproj(c):
                for i in range(4 * c, 4 * c + 4):
                    qq = slice((i % 4) * P, (i % 4 + 1) * P)
                    ot = phd.tile([P, D], dt.bfloat16, tag="outsb", name="ot")
                    for dc in range(NCH):
                        po = psp.tile([P, CH], dt.float32, tag="od", bufs=2,
                                      name="ps_o")
                        dsl = slice(dc * CH, (dc + 1) * CH)
                        nc.tensor.matmul(po, aT_tiles[(0, c)][:, qq],
                                         wo[:, 0, dsl], start=True, stop=False)
                        nc.tensor.matmul(po, aT_tiles[(1, c)][:, qq],
                                         wo[:, 1, dsl], start=False, stop=True)
                        nc.vector.tensor_copy(ot[:, dsl], po)
                    nc.sync.dma_start(out=out_d[i * P:(i + 1) * P, :], in_=ot)
                aT_tiles.pop((0, c))
                aT_tiles.pop((1, c))

            # software pipeline: emit scores one (c, h) block ahead of AV so
            # the PE never stalls on the scalar engine's exps; the output
            # projection for chunk c rides along after its last head.
            blocks = [(c, h) for c in range(NCH) for h in range(HLOC)]
            emit_scores(*blocks[0])
            for bi, (c, h) in enumerate(blocks):
                if bi + 1 < len(blocks):
                    emit_scores(*blocks[bi + 1])
                emit_av(c, h)
                if h == HLOC - 1:
                    emit_oproj(c)

    _split_sync_waits(nc)
    return nc


def kernel(**inputs):
    global LAST_RESULT
    inp = {k: np.asarray(v) for k, v in inputs.items()}
    h = inp["hidden_states"].astype(F32).reshape(S, D)
    mask = inp["attention_mask"].astype(F32).reshape(S, S)
    cos = inp["cos"].astype(F32)
    sin = inp["sin"].astype(F32)
    Wf = inp["Wf"].astype(F32)
    W1 = inp["W1"].astype(F32)
    b1 = inp["b1"].astype(F32)
    W2 = inp["W2"].astype(F32)
    b2 = float(inp["b2"].reshape(-1)[0])
    gate_scale = float(inp["gate_scale"])
    Wq = inp["Wq"].astype(F32)
    Wk = inp["Wk"].astype(F32)
    Wv = inp["Wv"].astype(F32)
    Wo = inp["Wo"].astype(F32)

    maskT = np.ascontiguousarray(mask.T)
    mb, patterns, av_incl = _analyze_mask(maskT)
    n_pat = len(patterns)
    assert n_pat <= 64, f"too many unique mask patterns ({n_pat})"

    field_scale = float(F32(1.0 - ALPHA))
    b2_scaled = float(F32(b2) * F32(field_scale))

    nc = _build_program(mb, n_pat, av_incl, field_scale, b2_scaled, gate_scale)

    # host-side shared tensors
    hT = np.ascontiguousarray(h.T).astype(BF16)
    cosT = np.ascontiguousarray(cos.T)                       # [64, S]
    sinT = np.ascontiguousarray(sin.T)
    sin_signed = sinT.copy()
    sin_signed[0:32] = -sin_signed[0:32]
    inv_sqrt_hd = 1.0 / math.sqrt(HD)
    cosq = (cosT * inv_sqrt_hd).astype(BF16)
    sinq = (sin_signed * inv_sqrt_hd).astype(BF16)
    cosk = cosT.astype(BF16)
    sink = sin_signed.astype(BF16)
    w1a = W1[:D].astype(BF16)
    w1b = W1[D:].astype(BF16)
    wf = Wf.astype(BF16)
    w2 = W2.reshape(64, 1).astype(BF16)
    b1c = b1.reshape(64, 1).astype(F32)
    pm = np.stack(patterns) if n_pat else None

    in_maps = []
    for c in range(NCORES):
        m = {
            "hT": hT,
            "wq": Wq[:, c * HLOC * HD:(c + 1) * HLOC * HD].astype(BF16),
            "wk": Wk[:, c * HD:(c + 1) * HD].astype(BF16),
            "wv": Wv[:, c * HD:(c + 1) * HD].astype(BF16),
            "wo": Wo[c * HLOC * HD:(c + 1) * HLOC * HD, :].astype(BF16),
            "w1a": w1a, "w1b": w1b, "wf": wf, "w2": w2, "b1": b1c,
            "cosq": cosq, "sinq": sinq, "cosk": cosk, "sink": sink,
        }
        if n_pat:
            m["pmask"] = pm
        in_maps.append(m)

    trace = False
    if os.environ.get("KERNEL_TRACE"):
        try:
            import antenv.axon_hooks  # noqa: F401  (profiling shim, dev only)
            trace = True
        except ImportError:
            pass

    res = run_bass_kernel_spmd(nc, in_maps, list(range(NCORES)), trace=trace)
    LAST_RESULT = res

    out = np.zeros((S, D), dtype=F32)
    for c in range(NCORES):
        out += res.results[c]["out"].astype(F32)
    return out.reshape(1, S, D)


# revision 39
# speedup vs baseline: 1.3607x; 1.3607x over previous
"""Trainium2 Bass kernel for nn_CFHoTWrapper (sparse attention with adapter gate).

Sharding: tensor-parallel over attention heads across 8 NeuronCores.
Each core computes 4 query heads + its 1 KV head end-to-end (QKV proj,
RoPE, scores, softmax, AV, partial O-projection); the tiny adapter gate
is replicated on every core. Per-core partial outputs (bf16) are summed
on the host.

Softmax is computed without max-subtraction (scores are O(5) for these
shapes so exp() is safe in fp32), and the per-key gate bias is folded in
multiplicatively: exp(s + m + g[k]) = exp(s) * exp(m) * w[k] with
w = exp(gate_scale * gate).  w scales the V rows, and an extra all-w
column appended to V yields the softmax denominator from the same
matmul that computes the numerator.
"""

import math
import os
from contextlib import ExitStack

import numpy as np
import ml_dtypes

import concourse.bass as bass
import concourse.tile as tile
from concourse import mybir
from concourse.masks import make_identity
from concourse.bass_utils import run_bass_kernel_spmd

BF16 = ml_dtypes.bfloat16
F32 = np.float32

S = 2048
D = 2048
HD = 64
NH = 32
NKV = 8
NCORES = 8
HLOC = NH // NCORES          # 4 query heads per core
P = 128
NT = S // P                  # 16 sequence tiles of 128
NCH = 4                      # 4 sequence chunks of 512
CH = 512
ALPHA = 0.995
MASK_NEG_THRESH = -80.0      # exp() underflows to 0 below this

LAST_RESULT = None           # BassKernelResults of the last run (for test.py)


def _analyze_mask(maskT):
    """Classify [keys=128 x q=128] blocks of maskT and dedup non-trivial
    multiplicative (exp) mask patterns. maskT is [S, S] (keys, q).

    Returns:
      mb:       [NT][NT] block class: 'skip' | 'plain' | int pattern id
      patterns: list of [128, 128] bf16 multiplicative masks
      av_incl:  per q-tile i, the key-tiles j contributing to softmax/AV
    """
    mb = [[None] * NT for _ in range(NT)]
    patterns = []
    pat_index = {}
    for j in range(NT):
        for i in range(NT):
            blk = maskT[j * P:(j + 1) * P, i * P:(i + 1) * P]
            if (blk < MASK_NEG_THRESH).all():
                mb[j][i] = 'skip'
            elif (blk == 0.0).all():
                mb[j][i] = 'plain'
            else:
                pat = np.exp(np.minimum(blk, 80.0)).astype(BF16)
                key = pat.tobytes()
                if key not in pat_index:
                    pat_index[key] = len(patterns)
                    patterns.append(pat)
                mb[j][i] = pat_index[key]
    av_incl = [[j for j in range(NT) if mb[j][i] != 'skip'] for i in range(NT)]
    return mb, patterns, av_incl


def _split_sync_waits(nc):
    """This walrus build supports only ONE embedded sync wait per
    instruction; hoist extra waits onto preceding sequencer NoOps."""
    for f in nc.m.functions:
        for bb in f.blocks:
            insts = bb.instructions
            idx = 0
            while idx < len(insts):
                inst = insts[idx]
                si = inst.sync_info
                if si is not None and si.on_wait and len(si.on_wait) > 1:
                    waits = list(si.on_wait)
                    for w in waits[:-1]:
                        nop = mybir.InstNoOp(
                            name=nc.get_next_instruction_name(),
                            engine=inst.engine,
                            sync_info=mybir.SyncInfo(on_wait=[w], on_update=[]),
                            bass_nofuse=True,
                        )
                        nc.register_instruction(nop)
                        insts.insert(idx, nop)
                        idx += 1
                    inst.sync_info = mybir.SyncInfo(
                        on_wait=[waits[-1]], on_update=list(si.on_update))
                idx += 1


def _build_program(mb, n_pat, av_incl, field_scale, b2_scaled, gate_scale,
                   erf_fn=None, debug=False):
    nc = bass.Bass()
    dt = mybir.dt
    if erf_fn is None:
        erf_fn = mybir.ActivationFunctionType.Erf
    dbg = {}
    if debug:
        dbg["qt0"] = nc.declare_dram_parameter("d_qt0", [HD, S], dt.bfloat16, isOutput=True)
        dbg["kt"] = nc.declare_dram_parameter("d_kt", [HD, S], dt.bfloat16, isOutput=True)
        dbg["vaug"] = nc.declare_dram_parameter("d_vaug", [P, NT * (HD + 1)], dt.bfloat16, isOutput=True)
        dbg["wcol"] = nc.declare_dram_parameter("d_wcol", [P, NT], dt.float32, isOutput=True)
        dbg["gate"] = nc.declare_dram_parameter("d_gate", [1, S], dt.float32, isOutput=True)
        dbg["field"] = nc.declare_dram_parameter("d_field", [1, S], dt.float32, isOutput=True)
        dbg["hmT"] = nc.declare_dram_parameter("d_hmT", [64, S], dt.bfloat16, isOutput=True)
        dbg["fibT"] = nc.declare_dram_parameter("d_fibT", [16, S], dt.bfloat16, isOutput=True)
        dbg["attn"] = nc.declare_dram_parameter("d_attn", [P, NT * HLOC * HD], dt.bfloat16, isOutput=True)
        dbg["pt00"] = nc.declare_dram_parameter("d_pt00", [P, CH], dt.bfloat16, isOutput=True)
        dbg["pair0"] = nc.declare_dram_parameter("d_pair0", [P, S], dt.bfloat16, isOutput=True)

    hT_d = nc.declare_dram_parameter("hT", [D, S], dt.bfloat16, isOutput=False)
    wq_d = nc.declare_dram_parameter("wq", [D, HLOC * HD], dt.bfloat16, isOutput=False)
    wk_d = nc.declare_dram_parameter("wk", [D, HD], dt.bfloat16, isOutput=False)
    wv_d = nc.declare_dram_parameter("wv", [D, HD], dt.bfloat16, isOutput=False)
    wo_d = nc.declare_dram_parameter("wo", [HLOC * HD, D], dt.bfloat16, isOutput=False)
    w1a_d = nc.declare_dram_parameter("w1a", [D, 64], dt.bfloat16, isOutput=False)
    w1b_d = nc.declare_dram_parameter("w1b", [16, 64], dt.bfloat16, isOutput=False)
    wf_d = nc.declare_dram_parameter("wf", [D, 16], dt.bfloat16, isOutput=False)
    w2_d = nc.declare_dram_parameter("w2", [64, 1], dt.bfloat16, isOutput=False)
    b1_d = nc.declare_dram_parameter("b1", [64, 1], dt.float32, isOutput=False)
    cosq_d = nc.declare_dram_parameter("cosq", [HD, S], dt.bfloat16, isOutput=False)
    sinq_d = nc.declare_dram_parameter("sinq", [HD, S], dt.bfloat16, isOutput=False)
    cosk_d = nc.declare_dram_parameter("cosk", [HD, S], dt.bfloat16, isOutput=False)
    sink_d = nc.declare_dram_parameter("sink", [HD, S], dt.bfloat16, isOutput=False)
    if n_pat:
        pm_d = nc.declare_dram_parameter("pmask", [n_pat, P, P], dt.bfloat16, isOutput=False)
    out_d = nc.declare_dram_parameter("out", [S, D], dt.bfloat16, isOutput=True)

    with tile.TileContext(nc) as tc, ExitStack() as ctx:
        pers = ctx.enter_context(tc.tile_pool(name="pers", bufs=1))
        psp = ctx.enter_context(tc.tile_pool(name="psum", bufs=4, space="PSUM"))

        # ---------------- persistent loads ----------------
        # small adapter weights first so the PE can start within ~4us;
        # hT streams per-chunk (consumers unblock chunk by chunk); wo last
        # (only needed by the output projection).
        wf = pers.tile([P, NT, 16], dt.bfloat16)
        nc.sync.dma_start(out=wf, in_=wf_d.rearrange("(k p) f -> p k f", p=P))
        w1a = pers.tile([P, NT, 64], dt.bfloat16)
        nc.sync.dma_start(out=w1a, in_=w1a_d.rearrange("(k p) f -> p k f", p=P))
        w1b = pers.tile([16, 64], dt.bfloat16)
        nc.sync.dma_start(out=w1b, in_=w1b_d[:, :])
        w2 = pers.tile([64, 1], dt.bfloat16)
        nc.sync.dma_start(out=w2, in_=w2_d[:, :])
        b1 = pers.tile([64, 1], dt.float32)
        nc.sync.dma_start(out=b1, in_=b1_d[:, :])
        hT = pers.tile([P, NT, S], dt.bfloat16)
        for k in range(NT):
            nc.sync.dma_start(out=hT[:, k, :], in_=hT_d[k * P:(k + 1) * P, :])
        wq = pers.tile([P, NT, HLOC * HD], dt.bfloat16)
        nc.sync.dma_start(out=wq, in_=wq_d.rearrange("(k p) f -> p k f", p=P))
        wk = pers.tile([P, NT, HD], dt.bfloat16)
        nc.sync.dma_start(out=wk, in_=wk_d.rearrange("(k p) f -> p k f", p=P))
        wv = pers.tile([P, NT, HD], dt.bfloat16)
        nc.sync.dma_start(out=wv, in_=wv_d.rearrange("(k p) f -> p k f", p=P))
        if n_pat:
            pmask = pers.tile([P, n_pat, P], dt.bfloat16)
            for m in range(n_pat):
                nc.sync.dma_start(out=pmask[:, m, :], in_=pm_d[m, :, :])
        wo = pers.tile([P, 2, D], dt.bfloat16)
        nc.sync.dma_start(out=wo, in_=wo_d.rearrange("(g p) d -> p g d", p=P))
        ident = pers.tile([P, P], dt.bfloat16)
        make_identity(nc, ident)
        ones64 = pers.tile([1, HD], dt.float32)
        nc.vector.memset(ones64, 1.0)

        qt = [pers.tile([HD, S], dt.bfloat16, tag=f"qt{h}", name=f"qt{h}")
              for h in range(HLOC)]
        kt = pers.tile([HD, S], dt.bfloat16)
        vaug = pers.tile([P, NT, HD + 1], dt.bfloat16)
        wcol = pers.tile([P, NT], dt.float32)

        # ---------------- phase A+B: adapter gate & QKV projections ----------
        with tc.tile_pool(name="phb", bufs=1) as phb, \
             tc.tile_pool(name="phbw", bufs=2) as phbw:
            cosq = phb.tile([HD, S], dt.bfloat16)
            nc.sync.dma_start(out=cosq, in_=cosq_d[:, :])
            sinq = phb.tile([HD, S], dt.bfloat16)
            nc.sync.dma_start(out=sinq, in_=sinq_d[:, :])
            cosk = phb.tile([HD, S], dt.bfloat16)
            nc.sync.dma_start(out=cosk, in_=cosk_d[:, :])
            sink = phb.tile([HD, S], dt.bfloat16)
            nc.sync.dma_start(out=sink, in_=sink_d[:, :])

            # --- adapter: fiberT = Wf^T @ hT ---
            fibT = phb.tile([16, S], dt.bfloat16)
            for c in range(NCH):
                ps = psp.tile([16, CH], dt.float32, tag="sc", bufs=4)
                for k in range(NT):
                    nc.tensor.matmul(ps, wf[:, k, :], hT[:, k, c * CH:(c + 1) * CH],
                                     start=(k == 0), stop=(k == NT - 1))
                nc.vector.tensor_copy(fibT[:, c * CH:(c + 1) * CH], ps)
            # --- hmidT = gelu(W1^T @ [hT; fibT] + b1) ---
            hmT = phb.tile([64, S], dt.bfloat16)
            for c in range(NCH):
                ps = psp.tile([64, CH], dt.float32, tag="sc", bufs=4)
                for k in range(NT):
                    nc.tensor.matmul(ps, w1a[:, k, :], hT[:, k, c * CH:(c + 1) * CH],
                                     start=(k == 0), stop=False)
                nc.tensor.matmul(ps, w1b, fibT[:, c * CH:(c + 1) * CH],
                                 start=False, stop=True)
                # exact gelu(x) = 0.5 * x * (1 + erf(x / sqrt(2))), x = ps + b1
                pre = phbw.tile([64, CH], dt.float32, tag="pre")
                nc.vector.tensor_scalar(pre, ps, b1, None, mybir.AluOpType.add)
                er = phbw.tile([64, CH], dt.float32, tag="er")
                nc.scalar.activation(er, pre, erf_fn,
                                     bias=0.0, scale=1.0 / math.sqrt(2.0))
                nc.vector.tensor_scalar(er, er, 0.5, 0.5,
                                        mybir.AluOpType.mult, mybir.AluOpType.add)
                nc.vector.tensor_mul(hmT[:, c * CH:(c + 1) * CH], pre, er)
            # --- field row = field_scale * (hmidT^T @ W2 + b2) ---
            field = phb.tile([1, S], dt.float32)
            scratch = phb.tile([1, S], dt.float32)
            for c in range(NCH):
                ps = psp.tile([1, CH], dt.float32, tag="sc", bufs=4)
                nc.tensor.matmul(ps, w2, hmT[:, c * CH:(c + 1) * CH],
                                 start=True, stop=True)
                nc.vector.tensor_scalar(field[:, c * CH:(c + 1) * CH], ps,
                                        field_scale, b2_scaled,
                                        mybir.AluOpType.mult, mybir.AluOpType.add)
            # --- standardize: gate = (field - mean) / (std_ddof1 + 1e-6) ---
            ssum = phb.tile([1, 1], dt.float32)
            nc.vector.reduce_sum(ssum, field, axis=mybir.AxisListType.X)
            mean = phb.tile([1, 1], dt.float32)
            nc.vector.tensor_scalar_mul(mean, ssum, 1.0 / S)
            nc.vector.tensor_scalar(field, field, mean, None, mybir.AluOpType.subtract)
            nc.scalar.square(scratch, field)
            ss2 = phb.tile([1, 1], dt.float32)
            nc.vector.reduce_sum(ss2, scratch, axis=mybir.AxisListType.X)
            std = phb.tile([1, 1], dt.float32)
            nc.scalar.activation(std, ss2, mybir.ActivationFunctionType.Sqrt,
                                 bias=0.0, scale=1.0 / (S - 1))
            nc.vector.tensor_scalar_add(std, std, 1e-6)
            rstd = phb.tile([1, 1], dt.float32)
            nc.vector.reciprocal(rstd, std)
            gsr = phb.tile([1, 1], dt.float32)
            nc.vector.tensor_scalar_mul(gsr, rstd, gate_scale)
            # w row = exp(gate_scale * gate), into scratch
            nc.scalar.activation(scratch, field, mybir.ActivationFunctionType.Exp,
                                 bias=0.0, scale=gsr)
            # transpose the w row into per-partition columns [128, 16] via a
            # DRAM bounce (SBUF partitions are not element-addressable across
            # the partition stride, so an in-SBUF gather is illegal on HW)
            wrow_dram = nc.dram_tensor("wrow_dram", [1, S], dt.float32)
            nc.sync.dma_start(out=wrow_dram[:, :], in_=scratch)
            nc.sync.dma_start(out=wcol,
                              in_=wrow_dram[0, :].rearrange("(j p) -> p j", p=P))
            if debug:
                nc.sync.dma_start(out=dbg["gate"][:, :], in_=scratch)
                nc.sync.dma_start(out=dbg["field"][:, :], in_=field)
                nc.sync.dma_start(out=dbg["hmT"][:, :], in_=hmT)
                nc.sync.dma_start(out=dbg["fibT"][:, :], in_=fibT)
                nc.sync.dma_start(out=dbg["wcol"][:, :], in_=wcol)

            # --- Q projection (head pairs) + RoPE ---
            # Compute engines require matching base partitions on SBUF
            # operands, so all partition moves (head extraction, the
            # rotate-half swap) go through SBUF->SBUF DMA.
            def rope(raw, swp, cos_t, sin_t, out_ap):
                t1 = phbw.tile([HD, S], dt.bfloat16, tag="t1")
                nc.vector.tensor_mul(t1, raw, cos_t)
                t2 = phbw.tile([HD, S], dt.bfloat16, tag="t2")
                nc.vector.tensor_mul(t2, swp, sin_t)
                nc.vector.tensor_add(out_ap, t1, t2)

            for t in range(HLOC // 2):
                pair = phbw.tile([P, S], dt.bfloat16, tag="qpair")
                for c in range(NCH):
                    ps = psp.tile([P, CH], dt.float32, tag="sc", bufs=4)
                    for k in range(NT):
                        nc.tensor.matmul(ps, wq[:, k, t * P:(t + 1) * P],
                                         hT[:, k, c * CH:(c + 1) * CH],
                                         start=(k == 0), stop=(k == NT - 1))
                    nc.vector.tensor_copy(pair[:, c * CH:(c + 1) * CH], ps)
                for hh in range(2):
                    h = 2 * t + hh
                    raw = phbw.tile([HD, S], dt.bfloat16, tag="qraw")
                    nc.sync.dma_start(out=raw, in_=pair[hh * HD:(hh + 1) * HD, :])
                    swp = phbw.tile([HD, S], dt.bfloat16, tag="qswp")
                    nc.sync.dma_start(out=swp[0:32, :],
                                      in_=pair[hh * HD + 32:hh * HD + 64, :])
                    nc.sync.dma_start(out=swp[32:64, :],
                                      in_=pair[hh * HD:hh * HD + 32, :])
                    rope(raw, swp, cosq, sinq, qt[h][:, :])
                    if debug and h == 0:
                        nc.sync.dma_start(out=dbg["pair0"][:, :], in_=pair)
                        nc.sync.dma_start(out=dbg["qt0"][:, :], in_=qt[0])
            # --- K projection + RoPE ---
            kraw = phbw.tile([HD, S], dt.bfloat16, tag="qraw")
            for c in range(NCH):
                ps = psp.tile([HD, CH], dt.float32, tag="sc", bufs=4)
                for k in range(NT):
                    nc.tensor.matmul(ps, wk[:, k, :], hT[:, k, c * CH:(c + 1) * CH],
                                     start=(k == 0), stop=(k == NT - 1))
                nc.vector.tensor_copy(kraw[:, c * CH:(c + 1) * CH], ps)
            kswp = phbw.tile([HD, S], dt.bfloat16, tag="qswp")
            nc.sync.dma_start(out=kswp[0:32, :], in_=kraw[32:64, :])
            nc.sync.dma_start(out=kswp[32:64, :], in_=kraw[0:32, :])
            rope(kraw, kswp, cosk, sink, kt[:, :])
            if debug:
                nc.sync.dma_start(out=dbg["kt"][:, :], in_=kt)
            # --- V projection (transposed form) + gate scaling + denom col ---
            vt = phb.tile([HD, S], dt.bfloat16)
            for c in range(NCH):
                ps = psp.tile([HD, CH], dt.float32, tag="sc", bufs=4)
                for k in range(NT):
                    nc.tensor.matmul(ps, wv[:, k, :], hT[:, k, c * CH:(c + 1) * CH],
                                     start=(k == 0), stop=(k == NT - 1))
                nc.vector.tensor_copy(vt[:, c * CH:(c + 1) * CH], ps)
            for st in range(NT):
                pv = psp.tile([P, HD], dt.bfloat16, tag="od", bufs=2)
                nc.tensor.transpose(pv, vt[:, st * P:(st + 1) * P],
                                    ident[0:HD, 0:HD])
                nc.vector.tensor_scalar(vaug[:, st, 0:HD], pv, wcol[:, st:st + 1],
                                        None, mybir.AluOpType.mult)
                nc.vector.tensor_copy(vaug[:, st, HD:HD + 1], wcol[:, st:st + 1])
            if debug:
                nc.sync.dma_start(
                    out=dbg["vaug"][:, :],
                    in_=vaug.rearrange("p a b -> p (a b)"))

        # ------------- phase C: attention + output projection -------------
        with tc.tile_pool(name="phc", bufs=2) as phc, \
             tc.tile_pool(name="phcs", bufs=4) as phcs, \
             tc.tile_pool(name="phd", bufs=3) as phd:

            # per-(j, chunk): first q-column (within the chunk) that isn't
            # fully masked -> scores MM / exp start column; None = skip tile
            def chunk_start(j, c):
                for ii in range(4 * c, 4 * c + 4):
                    if mb[j][ii] != 'skip':
                        return (ii % 4) * P
                return None

            pts_store = {}
            attn = phcs.tile([P, NT, HLOC * HD], dt.bfloat16, tag="attn", bufs=1)

            def emit_scores(c, h):
                pts = {}
                for j in range(NT):
                    s0 = chunk_start(j, c)
                    if s0 is None:
                        continue
                    ps = psp.tile([P, CH], dt.float32, tag="sc", bufs=4,
                                  name="ps_sc")
                    nc.tensor.matmul(ps[:, s0:CH], kt[:, j * P:(j + 1) * P],
                                     qt[h][:, c * CH + s0:(c + 1) * CH],
                                     start=True, stop=True)
                    pt = phc.tile([P, CH], dt.bfloat16, tag=f"pt{j}",
                                  name=f"pt{j}")
                    nc.scalar.activation(pt[:, s0:CH], ps[:, s0:CH],
                                         mybir.ActivationFunctionType.Exp)
                    for ii in range(4 * c, 4 * c + 4):
                        kind = mb[j][ii]
                        if kind in ('skip', 'plain'):
                            continue
                        qq = slice((ii % 4) * P, (ii % 4 + 1) * P)
                        nc.vector.tensor_mul(pt[:, qq], pt[:, qq],
                                             pmask[:, kind, :])
                    pts[j] = pt
                    if debug and h == 0 and c == 0 and j == 0:
                        nc.sync.dma_start(out=dbg["pt00"][:, :], in_=pt)
                pts_store[(c, h)] = pts

            def emit_av(c, h):
                pts = pts_store.pop((c, h))
                for i in range(4 * c, 4 * c + 4):
                    js = av_incl[i]
                    hsl = slice(h * HD, (h + 1) * HD)
                    if not js:
                        nc.vector.memset(attn[:, i, hsl], 0.0)
                        continue
                    pa = psp.tile([P, HD + 1], dt.float32, tag="av", bufs=2,
                                  name="ps_av")
                    qq = slice((i % 4) * P, (i % 4 + 1) * P)
                    for idx, j in enumerate(js):
                        nc.tensor.matmul(pa, pts[j][:, qq], vaug[:, j, :],
                                         start=(idx == 0),
                                         stop=(idx == len(js) - 1))
                    rc = phcs.tile([P, 1], dt.float32, tag="rc", name="rc")
                    nc.vector.reciprocal(rc, pa[:, HD:HD + 1])
                    nc.vector.tensor_scalar(attn[:, i, hsl], pa[:, 0:HD], rc,
                                            None, mybir.AluOpType.mult)

            def emit_oproj(c):
                for i in range(4 * c, 4 * c + 4):
                    aTs = []
                    for g in range(2):
                        ptr = psp.tile([P, P], dt.bfloat16, tag="od", bufs=2,
                                       name="ptr")
                        nc.tensor.transpose(ptr, attn[:, i, g * P:(g + 1) * P],
                                            ident)
                        aT = phcs.tile([P, P], dt.bfloat16, tag="aT", name="aT")
                        nc.vector.tensor_copy(aT, ptr)
                        aTs.append(aT)
                    ot = phd.tile([P, D], dt.bfloat16, tag="outsb", name="ot")
                    for dc in range(NCH):
                        po = psp.tile([P, CH], dt.float32, tag="od", bufs=2,
                                      name="ps_o")
                        dsl = slice(dc * CH, (dc + 1) * CH)
                        nc.tensor.matmul(po, aTs[0], wo[:, 0, dsl],
                                         start=True, stop=False)
                        nc.tensor.matmul(po, aTs[1], wo[:, 1, dsl],
                                         start=False, stop=True)
                        nc.vector.tensor_copy(ot[:, dsl], po)
                    nc.sync.dma_start(out=out_d[i * P:(i + 1) * P, :], in_=ot)

            # software pipeline: emit scores one (c, h) block ahead of AV so
            # the PE never stalls on the scalar engine's exps; the output
            # projection for chunk c rides along after its last head.
            blocks = [(c, h) for c in range(NCH) for h in range(HLOC)]
            emit_scores(*blocks[0])
            for bi, (c, h) in enumerate(blocks):
                if bi + 1 < len(blocks):
                    emit_scores(*blocks[bi + 1])
                emit_av(c, h)
                if h == HLOC - 1:
                    emit_oproj(c)

    _split_sync_waits(nc)
    return nc


def kernel(**inputs):
    global LAST_RESULT
    inp = {k: np.asarray(v) for k, v in inputs.items()}
    h = inp["hidden_states"].astype(F32).reshape(S, D)
    mask = inp["attention_mask"].astype(F32).reshape(S, S)
    cos = inp["cos"].astype(F32)
    sin = inp["sin"].astype(F32)
    Wf = inp["Wf"].astype(F32)
    W1 = inp["W1"].astype(F32)
    b1 = inp["b1"].astype(F32)
    W2 = inp["W2"].astype(F32)
    b2 = float(inp["b2"].reshape(-1)[0])
    gate_scale = float(inp["gate_scale"])
    Wq = inp["Wq"].astype(F32)
    Wk = inp["Wk"].astype(F32)
    Wv = inp["Wv"].astype(F32)
    Wo = inp["Wo"].astype(F32)

    maskT = np.ascontiguousarray(mask.T)
    mb, patterns, av_incl = _analyze_mask(maskT)
    n_pat = len(patterns)
    assert n_pat <= 64, f"too many unique mask patterns ({n_pat})"

    field_scale = float(F32(1.0 - ALPHA))
    b2_scaled = float(F32(b2) * F32(field_scale))

    nc = _build_program(mb, n_pat, av_incl, field_scale, b2_scaled, gate_scale)

    # host-side shared tensors
    hT = np.ascontiguousarray(h.T).astype(BF16)
    cosT = np.ascontiguousarray(cos.T)                       # [64, S]
    sinT = np.ascontiguousarray(sin.T)
    sin_signed = sinT.copy()
    sin_signed[0:32] = -sin_signed[0:32]
    inv_sqrt_hd = 1.0 / math.sqrt(HD)
    cosq = (cosT * inv_sqrt_hd).astype(BF16)
    sinq = (sin_signed * inv_sqrt_hd).astype(BF16)
    cosk = cosT.astype(BF16)
    sink = sin_signed.astype(BF16)
    w1a = W1[:D].astype(BF16)
    w1b = W1[D:].astype(BF16)
    wf = Wf.astype(BF16)
    w2 = W2.reshape(64, 1).astype(BF16)
    b1c = b1.reshape(64, 1).astype(F32)
    pm = np.stack(patterns) if n_pat else None

    in_maps = []
    for c in range(NCORES):
        m = {
            "hT": hT,
            "wq": Wq[:, c * HLOC * HD:(c + 1) * HLOC * HD].astype(BF16),
            "wk": Wk[:, c * HD:(c + 1) * HD].astype(BF16),
            "wv": Wv[:, c * HD:(c + 1) * HD].astype(BF16),
            "wo": Wo[c * HLOC * HD:(c + 1) * HLOC * HD, :].astype(BF16),
            "w1a": w1a, "w1b": w1b, "wf": wf, "w2": w2, "b1": b1c,
            "cosq": cosq, "sinq": sinq, "cosk": cosk, "sink": sink,
        }
        if n_pat:
            m["pmask"] = pm
        in_maps.append(m)

    trace = False
    if os.environ.get("KERNEL_TRACE"):
        try:
            import antenv.axon_hooks  # noqa: F401  (profiling shim, dev only)
            trace = True
        except ImportError:
            pass

    res = run_bass_kernel_spmd(nc, in_maps, list(range(NCORES)), trace=trace)
    LAST_RESULT = res

    out = np.zeros((S, D), dtype=F32)
    for c in range(NCORES):
        out += res.results[c]["out"].astype(F32)
    return out.reshape(1, S, D)


# revision 45
# speedup vs baseline: 1.4265x; 1.0484x over previous
"""Trainium2 Bass kernel for nn_CFHoTWrapper (sparse attention with adapter gate).

Sharding: tensor-parallel over attention heads across 8 NeuronCores.
Each core computes 4 query heads + its 1 KV head end-to-end (QKV proj,
RoPE, scores, softmax, AV, partial O-projection); the tiny adapter gate
is replicated on every core. Per-core partial outputs (bf16) are summed
on the host.

Softmax is computed without max-subtraction (scores are O(5) for these
shapes so exp() is safe in fp32), and the per-key gate bias is folded in
multiplicatively: exp(s + m + g[k]) = exp(s) * exp(m) * w[k] with
w = exp(gate_scale * gate).  w scales the V rows, and an extra all-w
column appended to V yields the softmax denominator from the same
matmul that computes the numerator.
"""

import math
import os
from contextlib import ExitStack

import numpy as np
import ml_dtypes

import concourse.bass as bass
import concourse.tile as tile
from concourse import mybir
from concourse.masks import make_identity
from concourse.bass_utils import run_bass_kernel_spmd

BF16 = ml_dtypes.bfloat16
F32 = np.float32

S = 2048
D = 2048
HD = 64
NH = 32
NKV = 8
NCORES = 8
HLOC = NH // NCORES          # 4 query heads per core
P = 128
NT = S // P                  # 16 sequence tiles of 128
NCH = 4                      # 4 sequence chunks of 512
CH = 512
ALPHA = 0.995
MASK_NEG_THRESH = -80.0      # exp() underflows to 0 below this

LAST_RESULT = None           # BassKernelResults of the last run (for test.py)


def _analyze_mask(maskT):
    """Classify [keys=128 x q=128] blocks of maskT and dedup non-trivial
    multiplicative (exp) mask patterns. maskT is [S, S] (keys, q).

    Returns:
      mb:       [NT][NT] block class: 'skip' | 'plain' | int pattern id
      patterns: list of [128, 128] bf16 multiplicative masks
      av_incl:  per q-tile i, the key-tiles j contributing to softmax/AV
    """
    mb = [[None] * NT for _ in range(NT)]
    patterns = []
    pat_index = {}
    for j in range(NT):
        for i in range(NT):
            blk = maskT[j * P:(j + 1) * P, i * P:(i + 1) * P]
            if (blk < MASK_NEG_THRESH).all():
                mb[j][i] = 'skip'
            elif (blk == 0.0).all():
                mb[j][i] = 'plain'
            else:
                pat = np.exp(np.minimum(blk, 80.0)).astype(BF16)
                key = pat.tobytes()
                if key not in pat_index:
                    pat_index[key] = len(patterns)
                    patterns.append(pat)
                mb[j][i] = pat_index[key]
    av_incl = [[j for j in range(NT) if mb[j][i] != 'skip'] for i in range(NT)]
    return mb, patterns, av_incl


def _split_sync_waits(nc):
    """This walrus build supports only ONE embedded sync wait per
    instruction; hoist extra waits onto preceding sequencer NoOps."""
    for f in nc.m.functions:
        for bb in f.blocks:
            insts = bb.instructions
            idx = 0
            while idx < len(insts):
                inst = insts[idx]
                si = inst.sync_info
                if si is not None and si.on_wait and len(si.on_wait) > 1:
                    waits = list(si.on_wait)
                    for w in waits[:-1]:
                        nop = mybir.InstNoOp(
                            name=nc.get_next_instruction_name(),
                            engine=inst.engine,
                            sync_info=mybir.SyncInfo(on_wait=[w], on_update=[]),
                            bass_nofuse=True,
                        )
                        nc.register_instruction(nop)
                        insts.insert(idx, nop)
                        idx += 1
                    inst.sync_info = mybir.SyncInfo(
                        on_wait=[waits[-1]], on_update=list(si.on_update))
                idx += 1


def _build_program(mb, n_pat, av_incl, field_scale, b2_scaled, gate_scale,
                   erf_fn=None, debug=False):
    nc = bass.Bass()
    dt = mybir.dt
    if erf_fn is None:
        erf_fn = mybir.ActivationFunctionType.Erf
    dbg = {}
    if debug:
        dbg["qt0"] = nc.declare_dram_parameter("d_qt0", [HD, S], dt.bfloat16, isOutput=True)
        dbg["kt"] = nc.declare_dram_parameter("d_kt", [HD, S], dt.bfloat16, isOutput=True)
        dbg["vaug"] = nc.declare_dram_parameter("d_vaug", [P, NT * (HD + 1)], dt.bfloat16, isOutput=True)
        dbg["wcol"] = nc.declare_dram_parameter("d_wcol", [P, NT], dt.float32, isOutput=True)
        dbg["gate"] = nc.declare_dram_parameter("d_gate", [1, S], dt.float32, isOutput=True)
        dbg["field"] = nc.declare_dram_parameter("d_field", [1, S], dt.float32, isOutput=True)
        dbg["hmT"] = nc.declare_dram_parameter("d_hmT", [64, S], dt.bfloat16, isOutput=True)
        dbg["fibT"] = nc.declare_dram_parameter("d_fibT", [16, S], dt.bfloat16, isOutput=True)
        dbg["attn"] = nc.declare_dram_parameter("d_attn", [P, NT * HLOC * HD], dt.bfloat16, isOutput=True)
        dbg["pt00"] = nc.declare_dram_parameter("d_pt00", [P, CH], dt.bfloat16, isOutput=True)
        dbg["pair0"] = nc.declare_dram_parameter("d_pair0", [P, S], dt.bfloat16, isOutput=True)

    hT_d = nc.declare_dram_parameter("hT", [D, S], dt.bfloat16, isOutput=False)
    wq_d = nc.declare_dram_parameter("wq", [D, HLOC * HD], dt.bfloat16, isOutput=False)
    wk_d = nc.declare_dram_parameter("wk", [D, HD], dt.bfloat16, isOutput=False)
    wv_d = nc.declare_dram_parameter("wv", [D, HD], dt.bfloat16, isOutput=False)
    wo_d = nc.declare_dram_parameter("wo", [HLOC * HD, D], dt.bfloat16, isOutput=False)
    w1a_d = nc.declare_dram_parameter("w1a", [D, 64], dt.bfloat16, isOutput=False)
    w1b_d = nc.declare_dram_parameter("w1b", [16, 64], dt.bfloat16, isOutput=False)
    wf_d = nc.declare_dram_parameter("wf", [D, 16], dt.bfloat16, isOutput=False)
    w2_d = nc.declare_dram_parameter("w2", [64, 1], dt.bfloat16, isOutput=False)
    b1_d = nc.declare_dram_parameter("b1", [64, 1], dt.float32, isOutput=False)
    cosq_d = nc.declare_dram_parameter("cosq", [HD, S], dt.bfloat16, isOutput=False)
    sinq_d = nc.declare_dram_parameter("sinq", [HD, S], dt.bfloat16, isOutput=False)
    cosk_d = nc.declare_dram_parameter("cosk", [HD, S], dt.bfloat16, isOutput=False)
    sink_d = nc.declare_dram_parameter("sink", [HD, S], dt.bfloat16, isOutput=False)
    if n_pat:
        pm_d = nc.declare_dram_parameter("pmask", [n_pat, P, P], dt.bfloat16, isOutput=False)
    out_d = nc.declare_dram_parameter("out", [S, D], dt.bfloat16, isOutput=True)

    with tile.TileContext(nc) as tc, ExitStack() as ctx:
        pers = ctx.enter_context(tc.tile_pool(name="pers", bufs=1))
        psp = ctx.enter_context(tc.tile_pool(name="psum", bufs=4, space="PSUM"))

        # ---------------- persistent loads ----------------
        # small adapter weights first so the PE can start within ~4us;
        # hT streams per-chunk (consumers unblock chunk by chunk); wo last
        # (only needed by the output projection).
        wf = pers.tile([P, NT, 16], dt.bfloat16)
        nc.sync.dma_start(out=wf, in_=wf_d.rearrange("(k p) f -> p k f", p=P))
        w1a = pers.tile([P, NT, 64], dt.bfloat16)
        nc.sync.dma_start(out=w1a, in_=w1a_d.rearrange("(k p) f -> p k f", p=P))
        w1b = pers.tile([16, 64], dt.bfloat16)
        nc.sync.dma_start(out=w1b, in_=w1b_d[:, :])
        w2 = pers.tile([64, 1], dt.bfloat16)
        nc.sync.dma_start(out=w2, in_=w2_d[:, :])
        b1 = pers.tile([64, 1], dt.float32)
        nc.sync.dma_start(out=b1, in_=b1_d[:, :])
        hT = pers.tile([P, NT, S], dt.bfloat16)
        for k in range(NT):
            nc.sync.dma_start(out=hT[:, k, :], in_=hT_d[k * P:(k + 1) * P, :])
        wq = pers.tile([P, NT, HLOC * HD], dt.bfloat16)
        nc.sync.dma_start(out=wq, in_=wq_d.rearrange("(k p) f -> p k f", p=P))
        wk = pers.tile([P, NT, HD], dt.bfloat16)
        nc.sync.dma_start(out=wk, in_=wk_d.rearrange("(k p) f -> p k f", p=P))
        wv = pers.tile([P, NT, HD], dt.bfloat16)
        nc.sync.dma_start(out=wv, in_=wv_d.rearrange("(k p) f -> p k f", p=P))
        if n_pat:
            pmask = pers.tile([P, n_pat, P], dt.bfloat16)
            for m in range(n_pat):
                nc.sync.dma_start(out=pmask[:, m, :], in_=pm_d[m, :, :])
        wo = pers.tile([P, 2, D], dt.bfloat16)
        nc.sync.dma_start(out=wo, in_=wo_d.rearrange("(g p) d -> p g d", p=P))
        ident = pers.tile([P, P], dt.bfloat16)
        make_identity(nc, ident)
        ones64 = pers.tile([1, HD], dt.float32)
        nc.vector.memset(ones64, 1.0)

        # head-pair Q tiles: head 2t in partitions 0:64, head 2t+1 in 64:128,
        # feeding two concurrent K=64 score matmuls in separate PE row-groups
        qt_pair = [pers.tile([P, S], dt.bfloat16, tag=f"qp{t}", name=f"qp{t}")
                   for t in range(HLOC // 2)]
        kt = pers.tile([HD, S], dt.bfloat16)
        ktp = pers.tile([P, S], dt.bfloat16)      # kt duplicated at base 64
        vaug = pers.tile([P, NT, HD + 1], dt.bfloat16)
        wcol = pers.tile([P, NT], dt.float32)

        # ---------------- phase A+B: adapter gate & QKV projections ----------
        with tc.tile_pool(name="phb", bufs=1) as phb, \
             tc.tile_pool(name="phbw", bufs=2) as phbw:
            cosq = phb.tile([HD, S], dt.bfloat16)
            nc.sync.dma_start(out=cosq, in_=cosq_d[:, :])
            sinq = phb.tile([HD, S], dt.bfloat16)
            nc.sync.dma_start(out=sinq, in_=sinq_d[:, :])
            cosk = phb.tile([HD, S], dt.bfloat16)
            nc.sync.dma_start(out=cosk, in_=cosk_d[:, :])
            sink = phb.tile([HD, S], dt.bfloat16)
            nc.sync.dma_start(out=sink, in_=sink_d[:, :])

            # --- adapter: fiberT = Wf^T @ hT ---
            fibT = phb.tile([16, S], dt.bfloat16)
            for c in range(NCH):
                ps = psp.tile([16, CH], dt.float32, tag="sc", bufs=4)
                for k in range(NT):
                    nc.tensor.matmul(ps, wf[:, k, :], hT[:, k, c * CH:(c + 1) * CH],
                                     start=(k == 0), stop=(k == NT - 1))
                nc.vector.tensor_copy(fibT[:, c * CH:(c + 1) * CH], ps)
            # --- hmidT = gelu(W1^T @ [hT; fibT] + b1) ---
            hmT = phb.tile([64, S], dt.bfloat16)
            for c in range(NCH):
                ps = psp.tile([64, CH], dt.float32, tag="sc", bufs=4)
                for k in range(NT):
                    nc.tensor.matmul(ps, w1a[:, k, :], hT[:, k, c * CH:(c + 1) * CH],
                                     start=(k == 0), stop=False)
                nc.tensor.matmul(ps, w1b, fibT[:, c * CH:(c + 1) * CH],
                                 start=False, stop=True)
                # exact gelu(x) = 0.5 * x * (1 + erf(x / sqrt(2))), x = ps + b1
                pre = phbw.tile([64, CH], dt.float32, tag="pre")
                nc.vector.tensor_scalar(pre, ps, b1, None, mybir.AluOpType.add)
                er = phbw.tile([64, CH], dt.float32, tag="er")
                nc.scalar.activation(er, pre, erf_fn,
                                     bias=0.0, scale=1.0 / math.sqrt(2.0))
                nc.vector.tensor_scalar(er, er, 0.5, 0.5,
                                        mybir.AluOpType.mult, mybir.AluOpType.add)
                nc.vector.tensor_mul(hmT[:, c * CH:(c + 1) * CH], pre, er)
            # --- field row = field_scale * (hmidT^T @ W2 + b2) ---
            field = phb.tile([1, S], dt.float32)
            scratch = phb.tile([1, S], dt.float32)
            for c in range(NCH):
                ps = psp.tile([1, CH], dt.float32, tag="sc", bufs=4)
                nc.tensor.matmul(ps, w2, hmT[:, c * CH:(c + 1) * CH],
                                 start=True, stop=True)
                nc.vector.tensor_scalar(field[:, c * CH:(c + 1) * CH], ps,
                                        field_scale, b2_scaled,
                                        mybir.AluOpType.mult, mybir.AluOpType.add)
            # --- standardize: gate = (field - mean) / (std_ddof1 + 1e-6) ---
            ssum = phb.tile([1, 1], dt.float32)
            nc.vector.reduce_sum(ssum, field, axis=mybir.AxisListType.X)
            mean = phb.tile([1, 1], dt.float32)
            nc.vector.tensor_scalar_mul(mean, ssum, 1.0 / S)
            nc.vector.tensor_scalar(field, field, mean, None, mybir.AluOpType.subtract)
            nc.scalar.square(scratch, field)
            ss2 = phb.tile([1, 1], dt.float32)
            nc.vector.reduce_sum(ss2, scratch, axis=mybir.AxisListType.X)
            std = phb.tile([1, 1], dt.float32)
            nc.scalar.activation(std, ss2, mybir.ActivationFunctionType.Sqrt,
                                 bias=0.0, scale=1.0 / (S - 1))
            nc.vector.tensor_scalar_add(std, std, 1e-6)
            rstd = phb.tile([1, 1], dt.float32)
            nc.vector.reciprocal(rstd, std)
            gsr = phb.tile([1, 1], dt.float32)
            nc.vector.tensor_scalar_mul(gsr, rstd, gate_scale)
            # w row = exp(gate_scale * gate), into scratch
            nc.scalar.activation(scratch, field, mybir.ActivationFunctionType.Exp,
                                 bias=0.0, scale=gsr)
            # transpose the w row into per-partition columns [128, 16] via a
            # DRAM bounce (SBUF partitions are not element-addressable across
            # the partition stride, so an in-SBUF gather is illegal on HW)
            wrow_dram = nc.dram_tensor("wrow_dram", [1, S], dt.float32)
            nc.sync.dma_start(out=wrow_dram[:, :], in_=scratch)
            nc.sync.dma_start(out=wcol,
                              in_=wrow_dram[0, :].rearrange("(j p) -> p j", p=P))
            if debug:
                nc.sync.dma_start(out=dbg["gate"][:, :], in_=scratch)
                nc.sync.dma_start(out=dbg["field"][:, :], in_=field)
                nc.sync.dma_start(out=dbg["hmT"][:, :], in_=hmT)
                nc.sync.dma_start(out=dbg["fibT"][:, :], in_=fibT)
                nc.sync.dma_start(out=dbg["wcol"][:, :], in_=wcol)

            # --- Q projection (head pairs) + RoPE ---
            # Compute engines require matching base partitions on SBUF
            # operands, so all partition moves (head extraction, the
            # rotate-half swap) go through SBUF->SBUF DMA.
            def rope(raw, swp, cos_t, sin_t, out_ap):
                t1 = phbw.tile([HD, S], dt.bfloat16, tag="t1", bufs=1)
                nc.vector.tensor_mul(t1, raw, cos_t)
                t2 = phbw.tile([HD, S], dt.bfloat16, tag="t2", bufs=1)
                nc.vector.tensor_mul(t2, swp, sin_t)
                nc.vector.tensor_add(out_ap, t1, t2)

            qt = [phb.tile([HD, S], dt.bfloat16, tag=f"qt{h}", name=f"qt{h}")
                  for h in range(HLOC)]
            for t in range(HLOC // 2):
                pair = phbw.tile([P, S], dt.bfloat16, tag="qpair", bufs=1)
                for c in range(NCH):
                    ps = psp.tile([P, CH], dt.float32, tag="sc", bufs=4)
                    for k in range(NT):
                        nc.tensor.matmul(ps, wq[:, k, t * P:(t + 1) * P],
                                         hT[:, k, c * CH:(c + 1) * CH],
                                         start=(k == 0), stop=(k == NT - 1))
                    nc.vector.tensor_copy(pair[:, c * CH:(c + 1) * CH], ps)
                for hh in range(2):
                    h = 2 * t + hh
                    raw = phbw.tile([HD, S], dt.bfloat16, tag="qraw")
                    nc.sync.dma_start(out=raw, in_=pair[hh * HD:(hh + 1) * HD, :])
                    swp = phbw.tile([HD, S], dt.bfloat16, tag="qswp")
                    nc.sync.dma_start(out=swp[0:32, :],
                                      in_=pair[hh * HD + 32:hh * HD + 64, :])
                    nc.sync.dma_start(out=swp[32:64, :],
                                      in_=pair[hh * HD:hh * HD + 32, :])
                    rope(raw, swp, cosq, sinq, qt[h][:, :])
                    nc.sync.dma_start(out=qt_pair[t][hh * HD:(hh + 1) * HD, :],
                                      in_=qt[h][:, :])
                    if debug and h == 0:
                        nc.sync.dma_start(out=dbg["pair0"][:, :], in_=pair)
                        nc.sync.dma_start(out=dbg["qt0"][:, :], in_=qt[0])
            # --- K projection + RoPE ---
            kraw = phbw.tile([HD, S], dt.bfloat16, tag="qraw")
            for c in range(NCH):
                ps = psp.tile([HD, CH], dt.float32, tag="sc", bufs=4)
                for k in range(NT):
                    nc.tensor.matmul(ps, wk[:, k, :], hT[:, k, c * CH:(c + 1) * CH],
                                     start=(k == 0), stop=(k == NT - 1))
                nc.vector.tensor_copy(kraw[:, c * CH:(c + 1) * CH], ps)
            kswp = phbw.tile([HD, S], dt.bfloat16, tag="qswp")
            nc.sync.dma_start(out=kswp[0:32, :], in_=kraw[32:64, :])
            nc.sync.dma_start(out=kswp[32:64, :], in_=kraw[0:32, :])
            rope(kraw, kswp, cosk, sink, kt[:, :])
            nc.sync.dma_start(out=ktp[HD:P, :], in_=kt[:, :])
            if debug:
                nc.sync.dma_start(out=dbg["kt"][:, :], in_=kt)
            # --- V projection (transposed form) + gate scaling + denom col ---
            vt = phb.tile([HD, S], dt.bfloat16)
            for c in range(NCH):
                ps = psp.tile([HD, CH], dt.float32, tag="sc", bufs=4)
                for k in range(NT):
                    nc.tensor.matmul(ps, wv[:, k, :], hT[:, k, c * CH:(c + 1) * CH],
                                     start=(k == 0), stop=(k == NT - 1))
                nc.vector.tensor_copy(vt[:, c * CH:(c + 1) * CH], ps)
            for st in range(NT):
                pv = psp.tile([P, HD], dt.bfloat16, tag="od", bufs=2)
                nc.tensor.transpose(pv, vt[:, st * P:(st + 1) * P],
                                    ident[0:HD, 0:HD])
                nc.vector.tensor_scalar(vaug[:, st, 0:HD], pv, wcol[:, st:st + 1],
                                        None, mybir.AluOpType.mult)
                nc.vector.tensor_copy(vaug[:, st, HD:HD + 1], wcol[:, st:st + 1])
            if debug:
                nc.sync.dma_start(
                    out=dbg["vaug"][:, :],
                    in_=vaug.rearrange("p a b -> p (a b)"))

        # ------------- phase C: attention + output projection -------------
        with tc.tile_pool(name="phc", bufs=2) as phc, \
             tc.tile_pool(name="phcs", bufs=4) as phcs, \
             tc.tile_pool(name="phd", bufs=3) as phd:

            # per-(j, chunk): first q-column (within the chunk) that isn't
            # fully masked -> scores MM / exp start column; None = skip tile
            def chunk_start(j, c):
                for ii in range(4 * c, 4 * c + 4):
                    if mb[j][ii] != 'skip':
                        return (ii % 4) * P
                return None

            pts_store = {}
            attn = phcs.tile([P, NT, HLOC * HD], dt.bfloat16, tag="attn", bufs=1)

            def emit_scores(c, t):
                """Scores for head pair (2t, 2t+1): two concurrent K=64
                matmuls in PE row-groups 0 and 64."""
                ptsA, ptsB = {}, {}
                for j in range(NT):
                    s0 = chunk_start(j, c)
                    if s0 is None:
                        continue
                    jsl = slice(j * P, (j + 1) * P)
                    csl = slice(c * CH + s0, (c + 1) * CH)
                    psA = psp.tile([P, CH], dt.float32, tag="sc", bufs=4,
                                   name="ps_scA")
                    nc.tensor.matmul(psA[:, s0:CH], kt[:, jsl],
                                     qt_pair[t][0:HD, csl],
                                     start=True, stop=True, tile_position=(0, 0))
                    psB = psp.tile([P, CH], dt.float32, tag="sc", bufs=4,
                                   name="ps_scB")
                    nc.tensor.matmul(psB[:, s0:CH], ktp[HD:P, jsl],
                                     qt_pair[t][HD:P, csl],
                                     start=True, stop=True, tile_position=(64, 0))
                    for pts, ps, tagc in ((ptsA, psA, "pt"), (ptsB, psB, "pu")):
                        pt = phc.tile([P, CH], dt.bfloat16, tag=f"{tagc}{j}",
                                      name=f"{tagc}{j}")
                        nc.scalar.activation(pt[:, s0:CH], ps[:, s0:CH],
                                             mybir.ActivationFunctionType.Exp)
                        for ii in range(4 * c, 4 * c + 4):
                            kind = mb[j][ii]
                            if kind in ('skip', 'plain'):
                                continue
                            qq = slice((ii % 4) * P, (ii % 4 + 1) * P)
                            nc.vector.tensor_mul(pt[:, qq], pt[:, qq],
                                                 pmask[:, kind, :])
                        pts[j] = pt
                pts_store[(c, 2 * t)] = ptsA
                pts_store[(c, 2 * t + 1)] = ptsB

            def emit_av(c, h):
                pts = pts_store.pop((c, h))
                for i in range(4 * c, 4 * c + 4):
                    js = av_incl[i]
                    hsl = slice(h * HD, (h + 1) * HD)
                    if not js:
                        nc.vector.memset(attn[:, i, hsl], 0.0)
                        continue
                    pa = psp.tile([P, HD + 1], dt.float32, tag="av", bufs=2,
                                  name="ps_av")
                    qq = slice((i % 4) * P, (i % 4 + 1) * P)
                    for idx, j in enumerate(js):
                        nc.tensor.matmul(pa, pts[j][:, qq], vaug[:, j, :],
                                         start=(idx == 0),
                                         stop=(idx == len(js) - 1))
                    rc = phcs.tile([P, 1], dt.float32, tag="rc", name="rc")
                    nc.vector.reciprocal(rc, pa[:, HD:HD + 1])
                    nc.vector.tensor_scalar(attn[:, i, hsl], pa[:, 0:HD], rc,
                                            None, mybir.AluOpType.mult)

            def emit_oproj(c):
                for i in range(4 * c, 4 * c + 4):
                    aTs = []
                    for g in range(2):
                        ptr = psp.tile([P, P], dt.bfloat16, tag="od", bufs=2,
                                       name="ptr")
                        nc.tensor.transpose(ptr, attn[:, i, g * P:(g + 1) * P],
                                            ident)
                        aT = phcs.tile([P, P], dt.bfloat16, tag="aT", name="aT")
                        nc.vector.tensor_copy(aT, ptr)
                        aTs.append(aT)
                    ot = phd.tile([P, D], dt.bfloat16, tag="outsb", name="ot")
                    for dc in range(NCH):
                        po = psp.tile([P, CH], dt.float32, tag="od", bufs=2,
                                      name="ps_o")
                        dsl = slice(dc * CH, (dc + 1) * CH)
                        nc.tensor.matmul(po, aTs[0], wo[:, 0, dsl],
                                         start=True, stop=False)
                        nc.tensor.matmul(po, aTs[1], wo[:, 1, dsl],
                                         start=False, stop=True)
                        nc.vector.tensor_copy(ot[:, dsl], po)
                    nc.sync.dma_start(out=out_d[i * P:(i + 1) * P, :], in_=ot)

            # software pipeline: emit scores one (c, pair) block ahead of AV
            # so the PE never stalls on the scalar engine's exps; the output
            # projection for chunk c rides along after its last head pair.
            blocks = [(c, t) for c in range(NCH) for t in range(HLOC // 2)]
            emit_scores(*blocks[0])
            for bi, (c, t) in enumerate(blocks):
                if bi + 1 < len(blocks):
                    emit_scores(*blocks[bi + 1])
                emit_av(c, 2 * t)
                emit_av(c, 2 * t + 1)
                if t == HLOC // 2 - 1:
                    emit_oproj(c)

    _split_sync_waits(nc)
    return nc


def kernel(**inputs):
    global LAST_RESULT
    inp = {k: np.asarray(v) for k, v in inputs.items()}
    h = inp["hidden_states"].astype(F32).reshape(S, D)
    mask = inp["attention_mask"].astype(F32).reshape(S, S)
    cos = inp["cos"].astype(F32)
    sin = inp["sin"].astype(F32)
    Wf = inp["Wf"].astype(F32)
    W1 = inp["W1"].astype(F32)
    b1 = inp["b1"].astype(F32)
    W2 = inp["W2"].astype(F32)
    b2 = float(inp["b2"].reshape(-1)[0])
    gate_scale = float(inp["gate_scale"])
    Wq = inp["Wq"].astype(F32)
    Wk = inp["Wk"].astype(F32)
    Wv = inp["Wv"].astype(F32)
    Wo = inp["Wo"].astype(F32)

    maskT = np.ascontiguousarray(mask.T)
    mb, patterns, av_incl = _analyze_mask(maskT)
    n_pat = len(patterns)
    assert n_pat <= 64, f"too many unique mask patterns ({n_pat})"

    field_scale = float(F32(1.0 - ALPHA))
    b2_scaled = float(F32(b2) * F32(field_scale))

    nc = _build_program(mb, n_pat, av_incl, field_scale, b2_scaled, gate_scale)

    # host-side shared tensors
    hT = np.ascontiguousarray(h.T).astype(BF16)
    cosT = np.ascontiguousarray(cos.T)                       # [64, S]
    sinT = np.ascontiguousarray(sin.T)
    sin_signed = sinT.copy()
    sin_signed[0:32] = -sin_signed[0:32]
    inv_sqrt_hd = 1.0 / math.sqrt(HD)
    cosq = (cosT * inv_sqrt_hd).astype(BF16)
    sinq = (sin_signed * inv_sqrt_hd).astype(BF16)
    cosk = cosT.astype(BF16)
    sink = sin_signed.astype(BF16)
    w1a = W1[:D].astype(BF16)
    w1b = W1[D:].astype(BF16)
    wf = Wf.astype(BF16)
    w2 = W2.reshape(64, 1).astype(BF16)
    b1c = b1.reshape(64, 1).astype(F32)
    pm = np.stack(patterns) if n_pat else None

    in_maps = []
    for c in range(NCORES):
        m = {
            "hT": hT,
            "wq": Wq[:, c * HLOC * HD:(c + 1) * HLOC * HD].astype(BF16),
            "wk": Wk[:, c * HD:(c + 1) * HD].astype(BF16),
            "wv": Wv[:, c * HD:(c + 1) * HD].astype(BF16),
            "wo": Wo[c * HLOC * HD:(c + 1) * HLOC * HD, :].astype(BF16),
            "w1a": w1a, "w1b": w1b, "wf": wf, "w2": w2, "b1": b1c,
            "cosq": cosq, "sinq": sinq, "cosk": cosk, "sink": sink,
        }
        if n_pat:
            m["pmask"] = pm
        in_maps.append(m)

    trace = False
    if os.environ.get("KERNEL_TRACE"):
        try:
            import antenv.axon_hooks  # noqa: F401  (profiling shim, dev only)
            trace = True
        except ImportError:
            pass

    res = run_bass_kernel_spmd(nc, in_maps, list(range(NCORES)), trace=trace)
    LAST_RESULT = res

    out = np.zeros((S, D), dtype=F32)
    for c in range(NCORES):
        out += res.results[c]["out"].astype(F32)
    return out.reshape(1, S, D)
